# revision 21
# baseline (speedup 1.0000x reference)
"""Trainium2 Bass kernel for llama-style attention block (B=4, S=1024, D=4096, H=32).

Strategy: tensor-parallel over heads across 8 NeuronCores (4 heads/core).
 - Host marshals inputs: x transposed to [D, T] (contraction dim on
   partitions), per-core weight slices pre-transposed, q/k weight rows
   deinterleaved (even/odd RoPE pairs -> partition blocks [0:64]/[64:128]),
   everything matmul-facing cast to bf16.
 - Device per core: QKV projections (PE, fp32 accum) -> RoPE (DVE) ->
   attention computed in transposed layout S^T[k,q] so softmax denominators
   come from an all-ones matmul and P@V needs no transposes -> per-batch
   AllGather of context (heads) -> output projection slice -> y columns.
 - Host concatenates the 8 per-core y column slices.
 - The program is specialized to the mask's tile structure (per 128x512
   score tile: all -inf -> skip entirely; all zero -> skip the additive
   mask; else general). Programs are cached per structure.

kernel(**inputs) takes the full unsharded inputs as in reference.setup_inputs()
and returns the full [4, 1024, 4096] float32 output.
"""

import math
import sys

import numpy as np
import ml_dtypes

sys.path.insert(0, "/opt/trn_rl_repo")

import concourse.bass as bass  # noqa: E402
import concourse.bass_isa as bass_isa  # noqa: E402
import concourse.mybir as mybir  # noqa: E402
import concourse.tile as tile  # noqa: E402
from concourse import bacc  # noqa: E402
from concourse.bass_utils import run_bass_kernel_spmd  # noqa: E402

P = 128
B, S, D, H = 4, 1024, 4096, 32
T = B * S
HD = 128
NCORES = 8
HPC = H // NCORES          # heads per core = 4
CW = HPC * HD              # per-core width = 512
NDK = D // P               # 32 contraction tiles
TCH = 512                  # token chunk in projection phase
NQ2 = S // 512             # q halves per batch
NKT = S // P               # 8 k tiles per batch

MM = mybir.dt.bfloat16     # matmul operand dtype
F32 = mybir.dt.float32
BF16 = ml_dtypes.bfloat16

AG_GROUPS = [list(range(NCORES))]


def build_program(mask_classes):
    """mask_classes[kt][q2] in {'d','z','g'}: dead / zero-add / general."""
    nc = bacc.Bacc("TRN2", target_bir_lowering=False, debug=False,
                   num_devices=NCORES)

    xT = nc.dram_tensor("xT", [D, T], MM, kind="ExternalInput").ap()
    wqT = nc.dram_tensor("wqT", [D, CW], MM, kind="ExternalInput").ap()
    wkT = nc.dram_tensor("wkT", [D, CW], MM, kind="ExternalInput").ap()
    wvT = nc.dram_tensor("wvT", [D, CW], MM, kind="ExternalInput").ap()
    woT = nc.dram_tensor("woT", [D, CW], MM, kind="ExternalInput").ap()
    maskT = nc.dram_tensor("maskT", [S, S], MM, kind="ExternalInput").ap()
    cq = nc.dram_tensor("cq", [HD // 2, S], F32, kind="ExternalInput").ap()
    sq = nc.dram_tensor("sq", [HD // 2, S], F32, kind="ExternalInput").ap()
    ck = nc.dram_tensor("ck", [HD // 2, S], F32, kind="ExternalInput").ap()
    sk = nc.dram_tensor("sk", [HD // 2, S], F32, kind="ExternalInput").ap()
    y = nc.dram_tensor("y", [T, CW], F32, kind="ExternalOutput").ap()

    qT_d = nc.dram_tensor("qT_d", [CW, T], MM).ap()
    kT_d = nc.dram_tensor("kT_d", [CW, T], MM).ap()
    v_d = nc.dram_tensor("v_d", [T, CW], MM).ap()
    # batch pairs share one AllGather: fewer collective syncs to pay for
    bounce = [nc.dram_tensor(f"bnc{i}", [CW, 2 * S], MM).ap() for i in range(2)]
    ctxT = [nc.dram_tensor(f"ctxT{i}", [D, 2 * S], MM, addr_space="Shared").ap()
            for i in range(2)]

    sub = mybir.AluOpType.subtract
    add = mybir.AluOpType.add
    mult = mybir.AluOpType.mult
    Exp = mybir.ActivationFunctionType.Exp

    # per q2: kt tiles that contribute (not dead)
    live_kt = [[kt for kt in range(NKT) if mask_classes[kt][q2] != 'd']
               for q2 in range(NQ2)]
    for q2 in range(NQ2):
        assert live_kt[q2], "fully-masked query block unsupported"

    with tile.TileContext(nc) as tc:
        # ---------------- Phase A: projections + RoPE ----------------
        with tc.tile_pool(name="wpool", bufs=1) as wpool, \
             tc.tile_pool(name="cspool", bufs=1) as cspool, \
             tc.tile_pool(name="xpool", bufs=2) as xpool, \
             tc.tile_pool(name="psa", bufs=4, space="PSUM") as psa, \
             tc.tile_pool(name="stga", bufs=4) as stga, \
             tc.tile_pool(name="tmpa", bufs=2) as tmpa:

            # first x chunk + head-0 q weights in interleaved pieces on two
            # DMA queues so the first accumulation group starts within ~5us
            x_first = xpool.tile([P, NDK, TCH], MM, tag="x")
            xTr = xT[:, 0:TCH].rearrange("(o p) t -> p o t", p=P)
            wq_sb = wpool.tile([P, NDK, CW], MM)
            wk_sb = wpool.tile([P, NDK, CW], MM)
            wv_sb = wpool.tile([P, NDK, CW], MM)
            wqr = wqT[:, 0:HD].rearrange("(o p) m -> p o m", p=P)
            wkr = wkT[:, 0:HD].rearrange("(o p) m -> p o m", p=P)
            for pc in range(8):
                dsl = slice(pc * 4, (pc + 1) * 4)
                nc.sync.dma_start(wq_sb[:, dsl, 0:HD], wqr[:, dsl, :])
                nc.scalar.dma_start(x_first[:, dsl, :], xTr[:, dsl, :])
            nc.sync.dma_start(wk_sb[:, :, 0:HD], wkr)

            cq_sb = cspool.tile([HD // 2, S], F32)
            sq_sb = cspool.tile([HD // 2, S], F32)
            ck_sb = cspool.tile([HD // 2, S], F32)
            sk_sb = cspool.tile([HD // 2, S], F32)
            nc.sync.dma_start(cq_sb, cq)
            nc.sync.dma_start(sq_sb, sq)
            nc.sync.dma_start(ck_sb, ck)
            nc.sync.dma_start(sk_sb, sk)

            for h in range(1, HPC):
                hs = slice(h * HD, (h + 1) * HD)
                nc.sync.dma_start(
                    wq_sb[:, :, hs],
                    wqT[:, hs].rearrange("(o p) m -> p o m", p=P))
                nc.sync.dma_start(
                    wk_sb[:, :, hs],
                    wkT[:, hs].rearrange("(o p) m -> p o m", p=P))
            nc.sync.dma_start(wv_sb, wvT.rearrange("(o p) m -> p o m", p=P))

            for tch in range(T // TCH):
                t0 = tch * TCH
                s0 = t0 % S
                if tch == 0:
                    x_sb = x_first
                else:
                    x_sb = xpool.tile([P, NDK, TCH], MM, tag="x")
                    nc.sync.dma_start(
                        x_sb,
                        xT[:, t0:t0 + TCH].rearrange("(o p) t -> p o t", p=P))

                # q/k for the 4 local heads; RoPE on psum eviction
                for h in range(HPC):
                    for wsb, c_sb, s_sb, dst in (
                            (wq_sb, cq_sb, sq_sb, qT_d),
                            (wk_sb, ck_sb, sk_sb, kT_d)):
                        ps = psa.tile([P, TCH], F32, tag="qk")
                        for dk in range(NDK):
                            nc.tensor.matmul(
                                ps, lhsT=wsb[:, dk, h * HD:(h + 1) * HD],
                                rhs=x_sb[:, dk, :],
                                start=(dk == 0), stop=(dk == NDK - 1))
                        a = ps[0:HD // 2]
                        bb = ps[HD // 2:P]
                        cc = c_sb[:, s0:s0 + TCH]
                        ss = s_sb[:, s0:s0 + TCH]
                        t1 = tmpa.tile([HD // 2, TCH], F32, tag="t1")
                        t2 = tmpa.tile([HD // 2, TCH], F32, tag="t2")
                        t3 = tmpa.tile([HD // 2, TCH], F32, tag="t3")
                        t4 = tmpa.tile([HD // 2, TCH], F32, tag="t4")
                        out = stga.tile([P, TCH], MM, tag="qkstage")
                        nc.vector.tensor_tensor(t1, a, cc, mult)
                        nc.vector.tensor_tensor(t2, bb, ss, mult)
                        nc.vector.tensor_tensor(out[0:HD // 2], t1, t2, sub)
                        nc.vector.tensor_tensor(t3, a, ss, mult)
                        nc.vector.tensor_tensor(t4, bb, cc, mult)
                        nc.vector.tensor_tensor(out[HD // 2:P], t3, t4, add)
                        nc.sync.dma_start(
                            dst[h * HD:(h + 1) * HD, t0:t0 + TCH], out)

                # v for the 4 local heads (natural [t, hd] layout);
                # evict on the otherwise-idle scalar engine
                for tt in range(TCH // P):
                    ps = psa.tile([P, CW], F32, tag="v")
                    for dk in range(NDK):
                        nc.tensor.matmul(
                            ps, lhsT=x_sb[:, dk, tt * P:(tt + 1) * P],
                            rhs=wv_sb[:, dk, :],
                            start=(dk == 0), stop=(dk == NDK - 1))
                    vo = stga.tile([P, CW], MM, tag="vstage")
                    nc.scalar.copy(vo, ps)
                    nc.sync.dma_start(
                        v_d[t0 + tt * P:t0 + (tt + 1) * P, :], vo)

        # ---------------- Phase B/C: attention + AllGather + wo ----------
        with tc.tile_pool(name="mpool", bufs=1) as mpool, \
             tc.tile_pool(name="qkvp", bufs=2) as qkvp, \
             tc.tile_pool(name="esp", bufs=3) as esp, \
             tc.tile_pool(name="psb", bufs=2, space="PSUM") as psb, \
             tc.tile_pool(name="tmpb", bufs=4) as tmpb, \
             tc.tile_pool(name="stgb", bufs=4) as stgb, \
             tc.tile_pool(name="cxp", bufs=2) as cxp:

            mask_sb = mpool.tile([P, NKT, S], MM)
            wo_sb = mpool.tile([P, NDK, CW], MM)

            def attn_batch(b):
                # whole-batch loads: one DMA per tensor covering all 4 heads
                qb = qkvp.tile([P, HPC, S], MM, tag="qb")
                kb = qkvp.tile([P, HPC, S], MM, tag="kb")
                vb = qkvp.tile([P, NKT, CW], MM, tag="vb")
                nc.sync.dma_start(
                    qb, qT_d[:, b * S:(b + 1) * S]
                    .rearrange("(h p) t -> p h t", p=P))
                nc.sync.dma_start(
                    kb, kT_d[:, b * S:(b + 1) * S]
                    .rearrange("(h p) t -> p h t", p=P))
                nc.sync.dma_start(
                    vb, v_d[b * S:(b + 1) * S, :]
                    .rearrange("(kt p) w -> p kt w", p=P))
                if b == 0:
                    nc.sync.dma_start(
                        mask_sb, maskT.rearrange("(kt p) q -> p kt q", p=P))
                # pass 1: scores + exp for all heads (PE runs ahead of ACT)
                es_h = []
                for h in range(HPC):
                    es = esp.tile([P, NKT, S], MM, tag="es")
                    es_h.append(es)
                    for kt in range(NKT):
                        for q2 in range(NQ2):
                            cls = mask_classes[kt][q2]
                            if cls == 'd':
                                continue
                            qsl = slice(q2 * 512, (q2 + 1) * 512)
                            ps_s = psb.tile([P, 512], F32, tag="sc", bufs=4)
                            nc.tensor.matmul(
                                ps_s, lhsT=kb[:, h, kt * P:(kt + 1) * P],
                                rhs=qb[:, h, qsl], start=True, stop=True)
                            if cls == 'z':
                                nc.scalar.activation(es[:, kt, qsl], ps_s, Exp)
                            else:
                                tmp = tmpb.tile([P, 512], F32, tag="sadd")
                                nc.vector.tensor_tensor(
                                    tmp, ps_s, mask_sb[:, kt, qsl], add)
                                nc.scalar.activation(es[:, kt, qsl], tmp, Exp)
                # pass 2: P@V + denominators (DVE k-sum + gpsimd partition
                # all-reduce, off the tensor engine) + normalize + bounce
                for h in range(HPC):
                    hs = slice(h * HD, (h + 1) * HD)
                    es = es_h[h]
                    for q2 in range(NQ2):
                        qsl = slice(q2 * 512, (q2 + 1) * 512)
                        lk = live_kt[q2]
                        ps_o = psb.tile([P, 512], F32, tag="ot", bufs=2)
                        for i, kt in enumerate(lk):
                            nc.tensor.matmul(
                                ps_o, lhsT=vb[:, kt, hs],
                                rhs=es[:, kt, qsl],
                                start=(i == 0), stop=(i == len(lk) - 1))
                        esum = tmpb.tile([P, 512], F32, tag="esum", bufs=2)
                        if len(lk) == 1:
                            nc.vector.tensor_copy(esum, es[:, lk[0], qsl])
                        else:
                            nc.vector.tensor_tensor(
                                esum, es[:, lk[0], qsl], es[:, lk[1], qsl],
                                add)
                            for kt in lk[2:]:
                                nc.vector.tensor_tensor(
                                    esum, esum, es[:, kt, qsl], add)
                        srep = tmpb.tile([P, 512], F32, tag="srep", bufs=2)
                        nc.gpsimd.partition_all_reduce(
                            srep, esum, channels=P,
                            reduce_op=bass_isa.ReduceOp.add)
                        rec = tmpb.tile([P, 512], F32, tag="rec", bufs=2)
                        nc.vector.reciprocal(rec, srep)
                        ob = stgb.tile([P, 512], MM, tag="ob", bufs=3)
                        nc.vector.tensor_tensor(ob, ps_o, rec, mult)
                        nc.sync.dma_start(
                            bounce[b // 2][h * HD:(h + 1) * HD,
                                           (b % 2) * S + q2 * 512:
                                           (b % 2) * S + (q2 + 1) * 512], ob)

            def wo_batch(b):
                # paired token tiles: 512B DMA lines on the ctx gather reads
                for tt in range(0, S // P, 2):
                    c0 = (b % 2) * S + tt * P
                    cx = cxp.tile([P, NDK, 2 * P], MM, tag="cx")
                    nc.sync.dma_start(
                        cx, ctxT[b // 2][:, c0:c0 + 2 * P]
                        .rearrange("(o p) t -> p o t", p=P))
                    ps_y0 = psb.tile([P, CW], F32, tag="y", bufs=2)
                    ps_y1 = psb.tile([P, CW], F32, tag="y", bufs=2)
                    for dk in range(NDK):
                        nc.tensor.matmul(
                            ps_y0, lhsT=cx[:, dk, 0:P], rhs=wo_sb[:, dk, :],
                            start=(dk == 0), stop=(dk == NDK - 1))
                        nc.tensor.matmul(
                            ps_y1, lhsT=cx[:, dk, P:2 * P], rhs=wo_sb[:, dk, :],
                            start=(dk == 0), stop=(dk == NDK - 1))
                    for j, ps_y in enumerate((ps_y0, ps_y1)):
                        yo = stgb.tile([P, CW], F32, tag="yo", bufs=2)
                        nc.scalar.copy(yo, ps_y)
                        nc.sync.dma_start(
                            y[b * S + (tt + j) * P:
                              b * S + (tt + j + 1) * P, :], yo)

            def allgather(i):
                nc.gpsimd.collective_compute(
                    "AllGather", mybir.AluOpType.bypass,
                    replica_groups=AG_GROUPS,
                    ins=[bounce[i]], outs=[ctxT[i]])

            # software-pipeline: AG(b0,b1) hidden under attention(b2),
            # AG(b2,b3) hidden under wo(b1)
            attn_batch(0)
            nc.sync.dma_start(wo_sb, woT.rearrange("(o p) m -> p o m", p=P))
            attn_batch(1)
            allgather(0)
            attn_batch(2)
            wo_batch(0)
            attn_batch(3)
            allgather(1)
            wo_batch(1)
            wo_batch(2)
            wo_batch(3)

    nc.compile()
    return nc


_NC_CACHE = {}


def _get_nc(mask_classes):
    key = tuple(map(tuple, mask_classes))
    if key not in _NC_CACHE:
        _NC_CACHE[key] = build_program(mask_classes)
    return _NC_CACHE[key]


def _classify_mask(maskT_f32):
    """Per score tile [kt*128:(kt+1)*128, q2*512:(q2+1)*512] of mask^T:
    'd' if fully -inf (softmax-dead), 'z' if all zero, else 'g'."""
    classes = []
    for kt in range(NKT):
        row = []
        for q2 in range(NQ2):
            t = maskT_f32[kt * P:(kt + 1) * P, q2 * 512:(q2 + 1) * 512]
            if np.all(t <= -1e30):
                row.append('d')
            elif np.all(t == 0.0):
                row.append('z')
            else:
                row.append('g')
        classes.append(row)
    return classes


def _prep_inputs(x, freqs_cos, freqs_sin, mask, wq, wk, wv, wo):
    """Host-side sharding/layout marshaling. Returns per-core input maps."""
    x = np.asarray(x, np.float32).reshape(T, D)
    xT = np.ascontiguousarray(x.T.astype(BF16))

    cos = np.asarray(freqs_cos, np.float32)
    sin = np.asarray(freqs_sin, np.float32)
    qscale = 1.0 / math.sqrt(HD)
    cqh = np.ascontiguousarray(cos.T * qscale).astype(np.float32)
    sqh = np.ascontiguousarray(sin.T * qscale).astype(np.float32)
    ckh = np.ascontiguousarray(cos.T).astype(np.float32)
    skh = np.ascontiguousarray(sin.T).astype(np.float32)

    m = np.asarray(mask, np.float32).reshape(S, S)
    mT = np.ascontiguousarray(m.T)
    classes = _classify_mask(mT)
    maskTb = np.ascontiguousarray(np.maximum(mT, -60000.0).astype(BF16))

    # deinterleave RoPE pairs within each head's weight rows: row order
    # [0,2,...,126,1,3,...,127] so pairs land in partition blocks.
    perm = np.concatenate([np.arange(0, HD, 2), np.arange(1, HD, 2)])

    wq = np.asarray(wq, np.float32)
    wk = np.asarray(wk, np.float32)
    wv = np.asarray(wv, np.float32)
    wo = np.asarray(wo, np.float32)

    in_maps = []
    for c in range(NCORES):
        r0, r1 = c * CW, (c + 1) * CW
        wq_c = wq[r0:r1].reshape(HPC, HD, D)[:, perm, :].reshape(CW, D)
        wk_c = wk[r0:r1].reshape(HPC, HD, D)[:, perm, :].reshape(CW, D)
        wv_c = wv[r0:r1]
        wo_c = wo[r0:r1]
        in_maps.append({
            "xT": xT,
            "wqT": np.ascontiguousarray(wq_c.T.astype(BF16)),
            "wkT": np.ascontiguousarray(wk_c.T.astype(BF16)),
            "wvT": np.ascontiguousarray(wv_c.T.astype(BF16)),
            "woT": np.ascontiguousarray(wo_c.T.astype(BF16)),
            "maskT": maskTb,
            "cq": cqh, "sq": sqh, "ck": ckh, "sk": skh,
        })
    return in_maps, classes


def kernel(x, start_pos, freqs_cos, freqs_sin, mask, wq, wk, wv, wo,
           cache_k, cache_v, _trace=False):
    assert int(start_pos) == 0, "kernel specialized for start_pos=0"
    in_maps, classes = _prep_inputs(x, freqs_cos, freqs_sin, mask,
                                    wq, wk, wv, wo)
    nc = _get_nc(classes)
    res = run_bass_kernel_spmd(nc, in_maps, list(range(NCORES)), trace=_trace)
    kernel.last_results = res
    yfull = np.concatenate([res.results[c]["y"] for c in range(NCORES)],
                           axis=1)
    return yfull.reshape(B, S, D).astype(np.float32)


# revision 25
# speedup vs baseline: 1.0616x; 1.0616x over previous
"""Trainium2 Bass kernel for llama-style attention block (B=4, S=1024, D=4096, H=32).

Strategy: tensor-parallel over heads across 8 NeuronCores (4 heads/core).
 - Host marshals inputs: x transposed to [D, T] (contraction dim on
   partitions), per-core weight slices pre-transposed, q/k weight rows
   deinterleaved (even/odd RoPE pairs -> partition blocks [0:64]/[64:128]),
   everything matmul-facing cast to bf16.
 - Device per core: QKV projections (PE, fp32 accum) -> RoPE (DVE) ->
   attention computed in transposed layout S^T[k,q] so softmax denominators
   come from an all-ones matmul and P@V needs no transposes -> per-batch
   AllGather of context (heads) -> output projection slice -> y columns.
 - Host concatenates the 8 per-core y column slices.
 - The program is specialized to the mask's tile structure (per 128x512
   score tile: all -inf -> skip entirely; all zero -> skip the additive
   mask; else general). Programs are cached per structure.

kernel(**inputs) takes the full unsharded inputs as in reference.setup_inputs()
and returns the full [4, 1024, 4096] float32 output.
"""

import math
import sys

import numpy as np
import ml_dtypes

sys.path.insert(0, "/opt/trn_rl_repo")

import concourse.bass as bass  # noqa: E402
import concourse.bass_isa as bass_isa  # noqa: E402
import concourse.mybir as mybir  # noqa: E402
import concourse.tile as tile  # noqa: E402
from concourse import bacc  # noqa: E402
from concourse.bass_utils import run_bass_kernel_spmd  # noqa: E402

P = 128
B, S, D, H = 4, 1024, 4096, 32
T = B * S
HD = 128
NCORES = 8
HPC = H // NCORES          # heads per core = 4
CW = HPC * HD              # per-core width = 512
NDK = D // P               # 32 contraction tiles
TCH = 512                  # token chunk in projection phase
NQ2 = S // 512             # q halves per batch
NKT = S // P               # 8 k tiles per batch

MM = mybir.dt.bfloat16     # matmul operand dtype
F32 = mybir.dt.float32
BF16 = ml_dtypes.bfloat16

AG_GROUPS = [list(range(NCORES))]


def build_program(mask_classes):
    """mask_classes[kt][q2] in {'d','z','g'}: dead / zero-add / general."""
    nc = bacc.Bacc("TRN2", target_bir_lowering=False, debug=False,
                   num_devices=NCORES)

    xT = nc.dram_tensor("xT", [D, T], MM, kind="ExternalInput").ap()
    wqT = nc.dram_tensor("wqT", [D, CW], MM, kind="ExternalInput").ap()
    wkT = nc.dram_tensor("wkT", [D, CW], MM, kind="ExternalInput").ap()
    wvT = nc.dram_tensor("wvT", [D, CW], MM, kind="ExternalInput").ap()
    woT = nc.dram_tensor("woT", [D, CW], MM, kind="ExternalInput").ap()
    maskT = nc.dram_tensor("maskT", [S, S], MM, kind="ExternalInput").ap()
    cq = nc.dram_tensor("cq", [HD // 2, S], F32, kind="ExternalInput").ap()
    sq = nc.dram_tensor("sq", [HD // 2, S], F32, kind="ExternalInput").ap()
    ck = nc.dram_tensor("ck", [HD // 2, S], F32, kind="ExternalInput").ap()
    sk = nc.dram_tensor("sk", [HD // 2, S], F32, kind="ExternalInput").ap()
    y = nc.dram_tensor("y", [T, CW], F32, kind="ExternalOutput").ap()

    qT_d = nc.dram_tensor("qT_d", [CW, T], MM).ap()
    kT_d = nc.dram_tensor("kT_d", [CW, T], MM).ap()
    v_d = nc.dram_tensor("v_d", [T, CW], MM).ap()
    # batch pairs share one AllGather: fewer collective syncs to pay for
    bounce = [nc.dram_tensor(f"bnc{i}", [CW, 2 * S], MM).ap() for i in range(2)]
    ctxT = [nc.dram_tensor(f"ctxT{i}", [D, 2 * S], MM, addr_space="Shared").ap()
            for i in range(2)]

    sub = mybir.AluOpType.subtract
    add = mybir.AluOpType.add
    mult = mybir.AluOpType.mult
    Exp = mybir.ActivationFunctionType.Exp

    # per q2: kt tiles that contribute (not dead)
    live_kt = [[kt for kt in range(NKT) if mask_classes[kt][q2] != 'd']
               for q2 in range(NQ2)]
    for q2 in range(NQ2):
        assert live_kt[q2], "fully-masked query block unsupported"

    with tile.TileContext(nc) as tc:
        # ---------------- Phase A: projections + RoPE ----------------
        with tc.tile_pool(name="wpool", bufs=1) as wpool, \
             tc.tile_pool(name="cspool", bufs=1) as cspool, \
             tc.tile_pool(name="xpool", bufs=2) as xpool, \
             tc.tile_pool(name="psa", bufs=4, space="PSUM") as psa, \
             tc.tile_pool(name="stga", bufs=4) as stga, \
             tc.tile_pool(name="tmpa", bufs=2) as tmpa:

            # first x chunk + head-0 q weights in interleaved pieces on two
            # DMA queues so the first accumulation group starts within ~5us
            x_first = xpool.tile([P, NDK, TCH], MM, tag="x")
            xTr = xT[:, 0:TCH].rearrange("(o p) t -> p o t", p=P)
            wq_sb = wpool.tile([P, NDK, CW], MM)
            wk_sb = wpool.tile([P, NDK, CW], MM)
            wv_sb = wpool.tile([P, NDK, CW], MM)
            wqr = wqT[:, 0:HD].rearrange("(o p) m -> p o m", p=P)
            wkr = wkT[:, 0:HD].rearrange("(o p) m -> p o m", p=P)
            for pc in range(8):
                dsl = slice(pc * 4, (pc + 1) * 4)
                nc.sync.dma_start(wq_sb[:, dsl, 0:HD], wqr[:, dsl, :])
                nc.scalar.dma_start(x_first[:, dsl, :], xTr[:, dsl, :])
            nc.sync.dma_start(wk_sb[:, :, 0:HD], wkr)

            cq_sb = cspool.tile([HD // 2, S], F32)
            sq_sb = cspool.tile([HD // 2, S], F32)
            ck_sb = cspool.tile([HD // 2, S], F32)
            sk_sb = cspool.tile([HD // 2, S], F32)
            nc.sync.dma_start(cq_sb, cq)
            nc.sync.dma_start(sq_sb, sq)
            nc.sync.dma_start(ck_sb, ck)
            nc.sync.dma_start(sk_sb, sk)

            for h in range(1, HPC):
                hs = slice(h * HD, (h + 1) * HD)
                nc.sync.dma_start(
                    wq_sb[:, :, hs],
                    wqT[:, hs].rearrange("(o p) m -> p o m", p=P))
                nc.sync.dma_start(
                    wk_sb[:, :, hs],
                    wkT[:, hs].rearrange("(o p) m -> p o m", p=P))
            nc.sync.dma_start(wv_sb, wvT.rearrange("(o p) m -> p o m", p=P))

            for tch in range(T // TCH):
                t0 = tch * TCH
                s0 = t0 % S
                if tch == 0:
                    x_sb = x_first
                else:
                    x_sb = xpool.tile([P, NDK, TCH], MM, tag="x")
                    nc.sync.dma_start(
                        x_sb,
                        xT[:, t0:t0 + TCH].rearrange("(o p) t -> p o t", p=P))

                # q/k for the 4 local heads; RoPE on psum eviction
                for h in range(HPC):
                    for wsb, c_sb, s_sb, dst in (
                            (wq_sb, cq_sb, sq_sb, qT_d),
                            (wk_sb, ck_sb, sk_sb, kT_d)):
                        ps = psa.tile([P, TCH], F32, tag="qk")
                        for dk in range(NDK):
                            nc.tensor.matmul(
                                ps, lhsT=wsb[:, dk, h * HD:(h + 1) * HD],
                                rhs=x_sb[:, dk, :],
                                start=(dk == 0), stop=(dk == NDK - 1))
                        a = ps[0:HD // 2]
                        bb = ps[HD // 2:P]
                        cc = c_sb[:, s0:s0 + TCH]
                        ss = s_sb[:, s0:s0 + TCH]
                        t1 = tmpa.tile([HD // 2, TCH], F32, tag="t1")
                        t2 = tmpa.tile([HD // 2, TCH], F32, tag="t2")
                        t3 = tmpa.tile([HD // 2, TCH], F32, tag="t3")
                        t4 = tmpa.tile([HD // 2, TCH], F32, tag="t4")
                        out = stga.tile([P, TCH], MM, tag="qkstage")
                        nc.vector.tensor_tensor(t1, a, cc, mult)
                        nc.vector.tensor_tensor(t2, bb, ss, mult)
                        nc.vector.tensor_tensor(out[0:HD // 2], t1, t2, sub)
                        nc.vector.tensor_tensor(t3, a, ss, mult)
                        nc.vector.tensor_tensor(t4, bb, cc, mult)
                        nc.vector.tensor_tensor(out[HD // 2:P], t3, t4, add)
                        nc.sync.dma_start(
                            dst[h * HD:(h + 1) * HD, t0:t0 + TCH], out)

                # v for the 4 local heads (natural [t, hd] layout);
                # evict on the otherwise-idle scalar engine
                for tt in range(TCH // P):
                    ps = psa.tile([P, CW], F32, tag="v")
                    for dk in range(NDK):
                        nc.tensor.matmul(
                            ps, lhsT=x_sb[:, dk, tt * P:(tt + 1) * P],
                            rhs=wv_sb[:, dk, :],
                            start=(dk == 0), stop=(dk == NDK - 1))
                    vo = stga.tile([P, CW], MM, tag="vstage")
                    nc.scalar.copy(vo, ps)
                    nc.sync.dma_start(
                        v_d[t0 + tt * P:t0 + (tt + 1) * P, :], vo)

        # ---------------- Phase B/C: attention + AllGather + wo ----------
        with tc.tile_pool(name="mpool", bufs=1) as mpool, \
             tc.tile_pool(name="qkvp", bufs=2) as qkvp, \
             tc.tile_pool(name="esp", bufs=3) as esp, \
             tc.tile_pool(name="psb", bufs=2, space="PSUM") as psb, \
             tc.tile_pool(name="tmpb", bufs=4) as tmpb, \
             tc.tile_pool(name="stgb", bufs=4) as stgb, \
             tc.tile_pool(name="cxp", bufs=2) as cxp:

            mask_sb = mpool.tile([P, NKT, S], MM)
            # all-ones stationary operand: the denominator matmul yields the
            # per-query softmax sum replicated across all 128 partitions
            ones_sb = mpool.tile([P, P], MM)
            nc.any.memset(ones_sb, 1.0)
            wo_sb = mpool.tile([P, NDK, CW], MM)

            def attn_batch(b):
                # whole-batch loads: one DMA per tensor covering all 4 heads
                qb = qkvp.tile([P, HPC, S], MM, tag="qb")
                kb = qkvp.tile([P, HPC, S], MM, tag="kb")
                vb = qkvp.tile([P, NKT, CW], MM, tag="vb")
                nc.sync.dma_start(
                    qb, qT_d[:, b * S:(b + 1) * S]
                    .rearrange("(h p) t -> p h t", p=P))
                nc.sync.dma_start(
                    kb, kT_d[:, b * S:(b + 1) * S]
                    .rearrange("(h p) t -> p h t", p=P))
                nc.sync.dma_start(
                    vb, v_d[b * S:(b + 1) * S, :]
                    .rearrange("(kt p) w -> p kt w", p=P))
                if b == 0:
                    nc.sync.dma_start(
                        mask_sb, maskT.rearrange("(kt p) q -> p kt q", p=P))
                # pass 1: scores + exp for all heads (PE runs ahead of ACT)
                es_h = []
                for h in range(HPC):
                    es = esp.tile([P, NKT, S], MM, tag="es")
                    es_h.append(es)
                    for kt in range(NKT):
                        for q2 in range(NQ2):
                            cls = mask_classes[kt][q2]
                            if cls == 'd':
                                continue
                            qsl = slice(q2 * 512, (q2 + 1) * 512)
                            ps_s = psb.tile([P, 512], F32, tag="sc", bufs=2)
                            nc.tensor.matmul(
                                ps_s, lhsT=kb[:, h, kt * P:(kt + 1) * P],
                                rhs=qb[:, h, qsl], start=True, stop=True)
                            if cls == 'z':
                                nc.scalar.activation(es[:, kt, qsl], ps_s, Exp)
                            else:
                                tmp = tmpb.tile([P, 512], F32, tag="sadd")
                                nc.vector.tensor_tensor(
                                    tmp, ps_s, mask_sb[:, kt, qsl], add)
                                nc.scalar.activation(es[:, kt, qsl], tmp, Exp)
                # pass 2: P@V + denominators (DVE k-sum + gpsimd partition
                # all-reduce, off the tensor engine) + normalize + bounce
                for h in range(HPC):
                    hs = slice(h * HD, (h + 1) * HD)
                    es = es_h[h]
                    for q2 in range(NQ2):
                        qsl = slice(q2 * 512, (q2 + 1) * 512)
                        lk = live_kt[q2]
                        ps_o = psb.tile([P, 512], F32, tag="ot", bufs=2)
                        for i, kt in enumerate(lk):
                            nc.tensor.matmul(
                                ps_o, lhsT=vb[:, kt, hs],
                                rhs=es[:, kt, qsl],
                                start=(i == 0), stop=(i == len(lk) - 1))
                        ps_m = psb.tile([P, 512], F32, tag="sum", bufs=2)
                        for i, kt in enumerate(lk):
                            nc.tensor.matmul(
                                ps_m, lhsT=ones_sb,
                                rhs=es[:, kt, qsl],
                                start=(i == 0), stop=(i == len(lk) - 1))
                        rec = tmpb.tile([P, 512], F32, tag="rec", bufs=2)
                        nc.vector.reciprocal(rec, ps_m)
                        ob = stgb.tile([P, 512], MM, tag="ob", bufs=3)
                        nc.vector.tensor_tensor(ob, ps_o, rec, mult)
                        nc.sync.dma_start(
                            bounce[b // 2][h * HD:(h + 1) * HD,
                                           (b % 2) * S + q2 * 512:
                                           (b % 2) * S + (q2 + 1) * 512], ob)

            def wo_batch(b):
                # paired token tiles: 512B DMA lines on the ctx gather reads
                for tt in range(0, S // P, 2):
                    c0 = (b % 2) * S + tt * P
                    cx = cxp.tile([P, NDK, 2 * P], MM, tag="cx")
                    nc.sync.dma_start(
                        cx, ctxT[b // 2][:, c0:c0 + 2 * P]
                        .rearrange("(o p) t -> p o t", p=P))
                    ps_y0 = psb.tile([P, CW], F32, tag="y", bufs=2)
                    ps_y1 = psb.tile([P, CW], F32, tag="y", bufs=2)
                    for dk in range(NDK):
                        nc.tensor.matmul(
                            ps_y0, lhsT=cx[:, dk, 0:P], rhs=wo_sb[:, dk, :],
                            start=(dk == 0), stop=(dk == NDK - 1))
                        nc.tensor.matmul(
                            ps_y1, lhsT=cx[:, dk, P:2 * P], rhs=wo_sb[:, dk, :],
                            start=(dk == 0), stop=(dk == NDK - 1))
                    for j, ps_y in enumerate((ps_y0, ps_y1)):
                        yo = stgb.tile([P, CW], F32, tag="yo", bufs=2)
                        nc.scalar.copy(yo, ps_y)
                        nc.sync.dma_start(
                            y[b * S + (tt + j) * P:
                              b * S + (tt + j + 1) * P, :], yo)

            def allgather(i):
                nc.gpsimd.collective_compute(
                    "AllGather", mybir.AluOpType.bypass,
                    replica_groups=AG_GROUPS,
                    ins=[bounce[i]], outs=[ctxT[i]])

            # software-pipeline: AG(b0,b1) hidden under attention(b2,b3),
            # AG(b2,b3) hidden under wo(b0,b1)
            attn_batch(0)
            nc.sync.dma_start(wo_sb, woT.rearrange("(o p) m -> p o m", p=P))
            attn_batch(1)
            allgather(0)
            attn_batch(2)
            attn_batch(3)
            allgather(1)
            wo_batch(0)
            wo_batch(1)
            wo_batch(2)
            wo_batch(3)

    nc.compile()
    return nc


_NC_CACHE = {}


def _get_nc(mask_classes):
    key = tuple(map(tuple, mask_classes))
    if key not in _NC_CACHE:
        _NC_CACHE[key] = build_program(mask_classes)
    return _NC_CACHE[key]


def _classify_mask(maskT_f32):
    """Per score tile [kt*128:(kt+1)*128, q2*512:(q2+1)*512] of mask^T:
    'd' if fully -inf (softmax-dead), 'z' if all zero, else 'g'."""
    classes = []
    for kt in range(NKT):
        row = []
        for q2 in range(NQ2):
            t = maskT_f32[kt * P:(kt + 1) * P, q2 * 512:(q2 + 1) * 512]
            if np.all(t <= -1e30):
                row.append('d')
            elif np.all(t == 0.0):
                row.append('z')
            else:
                row.append('g')
        classes.append(row)
    return classes


def _prep_inputs(x, freqs_cos, freqs_sin, mask, wq, wk, wv, wo):
    """Host-side sharding/layout marshaling. Returns per-core input maps."""
    x = np.asarray(x, np.float32).reshape(T, D)
    xT = np.ascontiguousarray(x.T.astype(BF16))

    cos = np.asarray(freqs_cos, np.float32)
    sin = np.asarray(freqs_sin, np.float32)
    qscale = 1.0 / math.sqrt(HD)
    cqh = np.ascontiguousarray(cos.T * qscale).astype(np.float32)
    sqh = np.ascontiguousarray(sin.T * qscale).astype(np.float32)
    ckh = np.ascontiguousarray(cos.T).astype(np.float32)
    skh = np.ascontiguousarray(sin.T).astype(np.float32)

    m = np.asarray(mask, np.float32).reshape(S, S)
    mT = np.ascontiguousarray(m.T)
    classes = _classify_mask(mT)
    maskTb = np.ascontiguousarray(np.maximum(mT, -60000.0).astype(BF16))

    # deinterleave RoPE pairs within each head's weight rows: row order
    # [0,2,...,126,1,3,...,127] so pairs land in partition blocks.
    perm = np.concatenate([np.arange(0, HD, 2), np.arange(1, HD, 2)])

    wq = np.asarray(wq, np.float32)
    wk = np.asarray(wk, np.float32)
    wv = np.asarray(wv, np.float32)
    wo = np.asarray(wo, np.float32)

    in_maps = []
    for c in range(NCORES):
        r0, r1 = c * CW, (c + 1) * CW
        wq_c = wq[r0:r1].reshape(HPC, HD, D)[:, perm, :].reshape(CW, D)
        wk_c = wk[r0:r1].reshape(HPC, HD, D)[:, perm, :].reshape(CW, D)
        wv_c = wv[r0:r1]
        wo_c = wo[r0:r1]
        in_maps.append({
            "xT": xT,
            "wqT": np.ascontiguousarray(wq_c.T.astype(BF16)),
            "wkT": np.ascontiguousarray(wk_c.T.astype(BF16)),
            "wvT": np.ascontiguousarray(wv_c.T.astype(BF16)),
            "woT": np.ascontiguousarray(wo_c.T.astype(BF16)),
            "maskT": maskTb,
            "cq": cqh, "sq": sqh, "ck": ckh, "sk": skh,
        })
    return in_maps, classes


def kernel(x, start_pos, freqs_cos, freqs_sin, mask, wq, wk, wv, wo,
           cache_k, cache_v, _trace=False):
    assert int(start_pos) == 0, "kernel specialized for start_pos=0"
    in_maps, classes = _prep_inputs(x, freqs_cos, freqs_sin, mask,
                                    wq, wk, wv, wo)
    nc = _get_nc(classes)
    res = run_bass_kernel_spmd(nc, in_maps, list(range(NCORES)), trace=_trace)
    kernel.last_results = res
    yfull = np.concatenate([res.results[c]["y"] for c in range(NCORES)],
                           axis=1)
    return yfull.reshape(B, S, D).astype(np.float32)


# revision 29
# speedup vs baseline: 1.1101x; 1.0458x over previous
"""Trainium2 Bass kernel for llama-style attention block (B=4, S=1024, D=4096, H=32).

Strategy: tensor-parallel over heads across 8 NeuronCores (4 heads/core).
 - Host marshals inputs: x transposed to [D, T] (contraction dim on
   partitions), per-core weight slices pre-transposed, q/k weight rows
   deinterleaved (even/odd RoPE pairs -> partition blocks [0:64]/[64:128]),
   everything matmul-facing cast to bf16.
 - Device per core: QKV projections (PE, fp32 accum) -> RoPE (DVE) ->
   attention computed in transposed layout S^T[k,q] so softmax denominators
   come from an all-ones matmul and P@V needs no transposes -> per-batch
   AllGather of context (heads) -> output projection slice -> y columns.
 - Host concatenates the 8 per-core y column slices.
 - The program is specialized to the mask's tile structure (per 128x512
   score tile: all -inf -> skip entirely; all zero -> skip the additive
   mask; else general). Programs are cached per structure.

kernel(**inputs) takes the full unsharded inputs as in reference.setup_inputs()
and returns the full [4, 1024, 4096] float32 output.
"""

import math
import sys

import numpy as np
import ml_dtypes

sys.path.insert(0, "/opt/trn_rl_repo")

import concourse.bass as bass  # noqa: E402
import concourse.bass_isa as bass_isa  # noqa: E402
import concourse.mybir as mybir  # noqa: E402
import concourse.tile as tile  # noqa: E402
from concourse import bacc  # noqa: E402
from concourse.bass_utils import run_bass_kernel_spmd  # noqa: E402

P = 128
B, S, D, H = 4, 1024, 4096, 32
T = B * S
HD = 128
NCORES = 8
HPC = H // NCORES          # heads per core = 4
CW = HPC * HD              # per-core width = 512
NDK = D // P               # 32 contraction tiles
TCH = 512                  # token chunk in projection phase
NQ2 = S // 512             # q halves per batch
NKT = S // P               # 8 k tiles per batch

MM = mybir.dt.bfloat16     # matmul operand dtype
F32 = mybir.dt.float32
BF16 = ml_dtypes.bfloat16

AG_GROUPS = [list(range(NCORES))]


def build_program(mask_classes):
    """mask_classes[kt][q2] in {'d','z','g'}: dead / zero-add / general."""
    nc = bacc.Bacc("TRN2", target_bir_lowering=False, debug=False,
                   num_devices=NCORES)

    xT = nc.dram_tensor("xT", [D, T], MM, kind="ExternalInput").ap()
    wqT = nc.dram_tensor("wqT", [D, CW], MM, kind="ExternalInput").ap()
    wkT = nc.dram_tensor("wkT", [D, CW], MM, kind="ExternalInput").ap()
    wvT = nc.dram_tensor("wvT", [D, CW], MM, kind="ExternalInput").ap()
    woT = nc.dram_tensor("woT", [D, CW], MM, kind="ExternalInput").ap()
    maskT = nc.dram_tensor("maskT", [S, S], MM, kind="ExternalInput").ap()
    cq = nc.dram_tensor("cq", [HD // 2, S], F32, kind="ExternalInput").ap()
    sq = nc.dram_tensor("sq", [HD // 2, S], F32, kind="ExternalInput").ap()
    ck = nc.dram_tensor("ck", [HD // 2, S], F32, kind="ExternalInput").ap()
    sk = nc.dram_tensor("sk", [HD // 2, S], F32, kind="ExternalInput").ap()
    y = nc.dram_tensor("y", [T, CW], F32, kind="ExternalOutput").ap()

    qT_d = nc.dram_tensor("qT_d", [CW, T], MM).ap()
    kT_d = nc.dram_tensor("kT_d", [CW, T], MM).ap()
    v_d = nc.dram_tensor("v_d", [T, CW], MM).ap()
    AG_SPLIT = 4  # collectives; each covers B // AG_SPLIT batches
    BPG = B // AG_SPLIT
    bounce = [nc.dram_tensor(f"bnc{i}", [CW, BPG * S], MM).ap()
              for i in range(AG_SPLIT)]
    ctxT = [nc.dram_tensor(f"ctxT{i}", [D, BPG * S], MM,
                           addr_space="Shared").ap()
            for i in range(AG_SPLIT)]

    sub = mybir.AluOpType.subtract
    add = mybir.AluOpType.add
    mult = mybir.AluOpType.mult
    Exp = mybir.ActivationFunctionType.Exp

    # per q2: kt tiles that contribute (not dead)
    live_kt = [[kt for kt in range(NKT) if mask_classes[kt][q2] != 'd']
               for q2 in range(NQ2)]
    for q2 in range(NQ2):
        assert live_kt[q2], "fully-masked query block unsupported"

    with tile.TileContext(nc) as tc:
        # ---------------- Phase A: projections + RoPE ----------------
        with tc.tile_pool(name="wpool", bufs=1) as wpool, \
             tc.tile_pool(name="cspool", bufs=1) as cspool, \
             tc.tile_pool(name="xpool", bufs=2) as xpool, \
             tc.tile_pool(name="psa", bufs=4, space="PSUM") as psa, \
             tc.tile_pool(name="stga", bufs=4) as stga, \
             tc.tile_pool(name="tmpa", bufs=2) as tmpa:

            # first x chunk + head-0 q weights in interleaved pieces on two
            # DMA queues so the first accumulation group starts within ~5us
            x_first = xpool.tile([P, NDK, TCH], MM, tag="x")
            xTr = xT[:, 0:TCH].rearrange("(o p) t -> p o t", p=P)
            wq_sb = wpool.tile([P, NDK, CW], MM)
            wk_sb = wpool.tile([P, NDK, CW], MM)
            wv_sb = wpool.tile([P, NDK, CW], MM)
            wqr = wqT[:, 0:HD].rearrange("(o p) m -> p o m", p=P)
            wkr = wkT[:, 0:HD].rearrange("(o p) m -> p o m", p=P)
            for pc in range(8):
                dsl = slice(pc * 4, (pc + 1) * 4)
                nc.sync.dma_start(wq_sb[:, dsl, 0:HD], wqr[:, dsl, :])
                nc.scalar.dma_start(x_first[:, dsl, :], xTr[:, dsl, :])
            nc.sync.dma_start(wk_sb[:, :, 0:HD], wkr)

            cq_sb = cspool.tile([HD // 2, S], F32)
            sq_sb = cspool.tile([HD // 2, S], F32)
            ck_sb = cspool.tile([HD // 2, S], F32)
            sk_sb = cspool.tile([HD // 2, S], F32)
            nc.sync.dma_start(cq_sb, cq)
            nc.sync.dma_start(sq_sb, sq)
            nc.sync.dma_start(ck_sb, ck)
            nc.sync.dma_start(sk_sb, sk)

            for h in range(1, HPC):
                hs = slice(h * HD, (h + 1) * HD)
                nc.sync.dma_start(
                    wq_sb[:, :, hs],
                    wqT[:, hs].rearrange("(o p) m -> p o m", p=P))
                nc.sync.dma_start(
                    wk_sb[:, :, hs],
                    wkT[:, hs].rearrange("(o p) m -> p o m", p=P))
            nc.sync.dma_start(wv_sb, wvT.rearrange("(o p) m -> p o m", p=P))

            for tch in range(T // TCH):
                t0 = tch * TCH
                s0 = t0 % S
                if tch == 0:
                    x_sb = x_first
                else:
                    x_sb = xpool.tile([P, NDK, TCH], MM, tag="x")
                    nc.sync.dma_start(
                        x_sb,
                        xT[:, t0:t0 + TCH].rearrange("(o p) t -> p o t", p=P))

                # q/k for the 4 local heads; RoPE on psum eviction
                for h in range(HPC):
                    for wsb, c_sb, s_sb, dst in (
                            (wq_sb, cq_sb, sq_sb, qT_d),
                            (wk_sb, ck_sb, sk_sb, kT_d)):
                        ps = psa.tile([P, TCH], F32, tag="qk")
                        for dk in range(NDK):
                            nc.tensor.matmul(
                                ps, lhsT=wsb[:, dk, h * HD:(h + 1) * HD],
                                rhs=x_sb[:, dk, :],
                                start=(dk == 0), stop=(dk == NDK - 1))
                        a = ps[0:HD // 2]
                        bb = ps[HD // 2:P]
                        cc = c_sb[:, s0:s0 + TCH]
                        ss = s_sb[:, s0:s0 + TCH]
                        t1 = tmpa.tile([HD // 2, TCH], F32, tag="t1")
                        t2 = tmpa.tile([HD // 2, TCH], F32, tag="t2")
                        t3 = tmpa.tile([HD // 2, TCH], F32, tag="t3")
                        t4 = tmpa.tile([HD // 2, TCH], F32, tag="t4")
                        out = stga.tile([P, TCH], MM, tag="qkstage")
                        nc.vector.tensor_tensor(t1, a, cc, mult)
                        nc.vector.tensor_tensor(t2, bb, ss, mult)
                        nc.vector.tensor_tensor(out[0:HD // 2], t1, t2, sub)
                        nc.vector.tensor_tensor(t3, a, ss, mult)
                        nc.vector.tensor_tensor(t4, bb, cc, mult)
                        nc.vector.tensor_tensor(out[HD // 2:P], t3, t4, add)
                        nc.sync.dma_start(
                            dst[h * HD:(h + 1) * HD, t0:t0 + TCH], out)

                # v for the 4 local heads (natural [t, hd] layout);
                # evict on the otherwise-idle scalar engine
                for tt in range(TCH // P):
                    ps = psa.tile([P, CW], F32, tag="v")
                    for dk in range(NDK):
                        nc.tensor.matmul(
                            ps, lhsT=x_sb[:, dk, tt * P:(tt + 1) * P],
                            rhs=wv_sb[:, dk, :],
                            start=(dk == 0), stop=(dk == NDK - 1))
                    vo = stga.tile([P, CW], MM, tag="vstage")
                    nc.scalar.copy(vo, ps)
                    nc.sync.dma_start(
                        v_d[t0 + tt * P:t0 + (tt + 1) * P, :], vo)

        # ---------------- Phase B/C: attention + AllGather + wo ----------
        with tc.tile_pool(name="mpool", bufs=1) as mpool, \
             tc.tile_pool(name="qkvp", bufs=2) as qkvp, \
             tc.tile_pool(name="esp", bufs=3) as esp, \
             tc.tile_pool(name="psb", bufs=2, space="PSUM") as psb, \
             tc.tile_pool(name="tmpb", bufs=4) as tmpb, \
             tc.tile_pool(name="stgb", bufs=4) as stgb, \
             tc.tile_pool(name="cxp", bufs=2) as cxp:

            mask_sb = mpool.tile([P, NKT, S], MM)
            # all-ones stationary operand: the denominator matmul yields the
            # per-query softmax sum replicated across all 128 partitions
            ones_sb = mpool.tile([P, P], MM)
            nc.any.memset(ones_sb, 1.0)
            wo_sb = mpool.tile([P, NDK, CW], MM)

            def attn_batch(b):
                # whole-batch loads: one DMA per tensor covering all 4 heads
                qb = qkvp.tile([P, HPC, S], MM, tag="qb")
                kb = qkvp.tile([P, HPC, S], MM, tag="kb")
                vb = qkvp.tile([P, NKT, CW], MM, tag="vb")
                nc.sync.dma_start(
                    qb, qT_d[:, b * S:(b + 1) * S]
                    .rearrange("(h p) t -> p h t", p=P))
                nc.sync.dma_start(
                    kb, kT_d[:, b * S:(b + 1) * S]
                    .rearrange("(h p) t -> p h t", p=P))
                nc.sync.dma_start(
                    vb, v_d[b * S:(b + 1) * S, :]
                    .rearrange("(kt p) w -> p kt w", p=P))
                if b == 0:
                    nc.sync.dma_start(
                        mask_sb, maskT.rearrange("(kt p) q -> p kt q", p=P))
                # pass 1: scores + exp for all heads (PE runs ahead of ACT)
                es_h = []
                for h in range(HPC):
                    es = esp.tile([P, NKT, S], MM, tag="es")
                    es_h.append(es)
                    for kt in range(NKT):
                        for q2 in range(NQ2):
                            cls = mask_classes[kt][q2]
                            if cls == 'd':
                                continue
                            qsl = slice(q2 * 512, (q2 + 1) * 512)
                            ps_s = psb.tile([P, 512], F32, tag="sc", bufs=2)
                            nc.tensor.matmul(
                                ps_s, lhsT=kb[:, h, kt * P:(kt + 1) * P],
                                rhs=qb[:, h, qsl], start=True, stop=True)
                            if cls == 'z':
                                nc.scalar.activation(es[:, kt, qsl], ps_s, Exp)
                            else:
                                tmp = tmpb.tile([P, 512], F32, tag="sadd")
                                nc.vector.tensor_tensor(
                                    tmp, ps_s, mask_sb[:, kt, qsl], add)
                                nc.scalar.activation(es[:, kt, qsl], tmp, Exp)
                # pass 2: P@V + denominators (DVE k-sum + gpsimd partition
                # all-reduce, off the tensor engine) + normalize + bounce
                for h in range(HPC):
                    hs = slice(h * HD, (h + 1) * HD)
                    es = es_h[h]
                    for q2 in range(NQ2):
                        qsl = slice(q2 * 512, (q2 + 1) * 512)
                        lk = live_kt[q2]
                        ps_o = psb.tile([P, 512], F32, tag="ot", bufs=2)
                        for i, kt in enumerate(lk):
                            nc.tensor.matmul(
                                ps_o, lhsT=vb[:, kt, hs],
                                rhs=es[:, kt, qsl],
                                start=(i == 0), stop=(i == len(lk) - 1))
                        ps_m = psb.tile([P, 512], F32, tag="sum", bufs=2)
                        for i, kt in enumerate(lk):
                            nc.tensor.matmul(
                                ps_m, lhsT=ones_sb,
                                rhs=es[:, kt, qsl],
                                start=(i == 0), stop=(i == len(lk) - 1))
                        rec = tmpb.tile([P, 512], F32, tag="rec", bufs=2)
                        nc.vector.reciprocal(rec, ps_m)
                        ob = stgb.tile([P, 512], MM, tag="ob", bufs=3)
                        nc.vector.tensor_tensor(ob, ps_o, rec, mult)
                        nc.sync.dma_start(
                            bounce[b // BPG][h * HD:(h + 1) * HD,
                                             (b % BPG) * S + q2 * 512:
                                             (b % BPG) * S + (q2 + 1) * 512],
                            ob)

            def wo_batch(b):
                # paired token tiles: 512B DMA lines on the ctx gather reads
                for tt in range(0, S // P, 2):
                    c0 = (b % BPG) * S + tt * P
                    cx = cxp.tile([P, NDK, 2 * P], MM, tag="cx")
                    nc.sync.dma_start(
                        cx, ctxT[b // BPG][:, c0:c0 + 2 * P]
                        .rearrange("(o p) t -> p o t", p=P))
                    ps_y0 = psb.tile([P, CW], F32, tag="y", bufs=2)
                    ps_y1 = psb.tile([P, CW], F32, tag="y", bufs=2)
                    for dk in range(NDK):
                        nc.tensor.matmul(
                            ps_y0, lhsT=cx[:, dk, 0:P], rhs=wo_sb[:, dk, :],
                            start=(dk == 0), stop=(dk == NDK - 1))
                        nc.tensor.matmul(
                            ps_y1, lhsT=cx[:, dk, P:2 * P], rhs=wo_sb[:, dk, :],
                            start=(dk == 0), stop=(dk == NDK - 1))
                    for j, ps_y in enumerate((ps_y0, ps_y1)):
                        yo = stgb.tile([P, CW], F32, tag="yo", bufs=2)
                        nc.scalar.copy(yo, ps_y)
                        nc.sync.dma_start(
                            y[b * S + (tt + j) * P:
                              b * S + (tt + j + 1) * P, :], yo)

            def allgather(i):
                nc.gpsimd.collective_compute(
                    "AllGather", mybir.AluOpType.bypass,
                    replica_groups=AG_GROUPS,
                    ins=[bounce[i]], outs=[ctxT[i]])

            # software-pipeline: per-batch AllGathers hidden under the
            # following attention/wo batches
            attn_batch(0)
            nc.sync.dma_start(wo_sb, woT.rearrange("(o p) m -> p o m", p=P))
            allgather(0)
            attn_batch(1)
            allgather(1)
            attn_batch(2)
            allgather(2)
            wo_batch(0)
            attn_batch(3)
            allgather(3)
            wo_batch(1)
            wo_batch(2)
            wo_batch(3)

    nc.compile()
    return nc


_NC_CACHE = {}


def _get_nc(mask_classes):
    key = tuple(map(tuple, mask_classes))
    if key not in _NC_CACHE:
        _NC_CACHE[key] = build_program(mask_classes)
    return _NC_CACHE[key]


def _classify_mask(maskT_f32):
    """Per score tile [kt*128:(kt+1)*128, q2*512:(q2+1)*512] of mask^T:
    'd' if fully -inf (softmax-dead), 'z' if all zero, else 'g'."""
    classes = []
    for kt in range(NKT):
        row = []
        for q2 in range(NQ2):
            t = maskT_f32[kt * P:(kt + 1) * P, q2 * 512:(q2 + 1) * 512]
            if np.all(t <= -1e30):
                row.append('d')
            elif np.all(t == 0.0):
                row.append('z')
            else:
                row.append('g')
        classes.append(row)
    return classes


def _prep_inputs(x, freqs_cos, freqs_sin, mask, wq, wk, wv, wo):
    """Host-side sharding/layout marshaling. Returns per-core input maps."""
    x = np.asarray(x, np.float32).reshape(T, D)
    xT = np.ascontiguousarray(x.T.astype(BF16))

    cos = np.asarray(freqs_cos, np.float32)
    sin = np.asarray(freqs_sin, np.float32)
    qscale = 1.0 / math.sqrt(HD)
    cqh = np.ascontiguousarray(cos.T * qscale).astype(np.float32)
    sqh = np.ascontiguousarray(sin.T * qscale).astype(np.float32)
    ckh = np.ascontiguousarray(cos.T).astype(np.float32)
    skh = np.ascontiguousarray(sin.T).astype(np.float32)

    m = np.asarray(mask, np.float32).reshape(S, S)
    mT = np.ascontiguousarray(m.T)
    classes = _classify_mask(mT)
    maskTb = np.ascontiguousarray(np.maximum(mT, -60000.0).astype(BF16))

    # deinterleave RoPE pairs within each head's weight rows: row order
    # [0,2,...,126,1,3,...,127] so pairs land in partition blocks.
    perm = np.concatenate([np.arange(0, HD, 2), np.arange(1, HD, 2)])

    wq = np.asarray(wq, np.float32)
    wk = np.asarray(wk, np.float32)
    wv = np.asarray(wv, np.float32)
    wo = np.asarray(wo, np.float32)

    in_maps = []
    for c in range(NCORES):
        r0, r1 = c * CW, (c + 1) * CW
        wq_c = wq[r0:r1].reshape(HPC, HD, D)[:, perm, :].reshape(CW, D)
        wk_c = wk[r0:r1].reshape(HPC, HD, D)[:, perm, :].reshape(CW, D)
        wv_c = wv[r0:r1]
        wo_c = wo[r0:r1]
        in_maps.append({
            "xT": xT,
            "wqT": np.ascontiguousarray(wq_c.T.astype(BF16)),
            "wkT": np.ascontiguousarray(wk_c.T.astype(BF16)),
            "wvT": np.ascontiguousarray(wv_c.T.astype(BF16)),
            "woT": np.ascontiguousarray(wo_c.T.astype(BF16)),
            "maskT": maskTb,
            "cq": cqh, "sq": sqh, "ck": ckh, "sk": skh,
        })
    return in_maps, classes


def kernel(x, start_pos, freqs_cos, freqs_sin, mask, wq, wk, wv, wo,
           cache_k, cache_v, _trace=False):
    assert int(start_pos) == 0, "kernel specialized for start_pos=0"
    in_maps, classes = _prep_inputs(x, freqs_cos, freqs_sin, mask,
                                    wq, wk, wv, wo)
    nc = _get_nc(classes)
    res = run_bass_kernel_spmd(nc, in_maps, list(range(NCORES)), trace=_trace)
    kernel.last_results = res
    yfull = np.concatenate([res.results[c]["y"] for c in range(NCORES)],
                           axis=1)
    return yfull.reshape(B, S, D).astype(np.float32)
